# revision 1
# baseline (speedup 1.0000x reference)
import sys

for _p in ("/opt/trn_rl_repo", "/root/.axon_site/_ro/trn_rl_repo"):
    if _p not in sys.path:
        sys.path.insert(0, _p)

import numpy as np
import ml_dtypes

# Problem constants (nn_LocalConvolution): x [4,256,64,64] f32,
# weight [4,1,16,49,64,64] f32, K=7, pad=3, stride=1, dil=1.
# out[b, g*16+cc, y, x] = sum_k x_pad[b, g*16+cc, y+kh-3, x+kw-3] * w[b,0,cc,k,y,x]
B, C, H, W = 4, 256, 64, 64
WC, KK, K, PAD = 16, 49, 7, 3
NCORES = 8
HHALF = H // 2  # 32 rows per core (B=4 x 2 H-halves = 8 shards)
PART = 128
NBLK = C // PART  # 2 channel blocks per core
XR, XC = HHALF + 2 * PAD, W + 2 * PAD  # 38 x 70 padded shard
BANK = 512  # fp32 elems per PSUM bank
NBANK = HHALF * W // BANK  # 4 banks per block
YB = BANK // W  # 8 y-rows per bank

_BF16 = ml_dtypes.bfloat16
_cache = {}


def _build():
    import concourse.bacc as bacc
    import concourse.mybir as mybir
    import concourse.tile as tile

    nc = bacc.Bacc(None, target_bir_lowering=False)
    bf = mybir.dt.bfloat16
    f32 = mybir.dt.float32

    xa_d = nc.dram_tensor("xa", (PART, NBLK * XR * XC), bf, kind="ExternalInput")
    xb_d = nc.dram_tensor("xb", (PART, NBLK * XR * XC), bf, kind="ExternalInput")
    wr_d = nc.dram_tensor("wr", (PART, KK * HHALF * W), bf, kind="ExternalInput")
    id_d = nc.dram_tensor("ident", (PART, PART), bf, kind="ExternalInput")
    out_d = nc.dram_tensor("out", (PART, NBLK * HHALF * W), f32, kind="ExternalOutput")

    with tile.TileContext(nc) as tc:
        with (
            tc.tile_pool(name="xpool", bufs=1) as xpool,
            tc.tile_pool(name="cpool", bufs=1) as cpool,
            tc.tile_pool(name="wpool", bufs=4) as wpool,
            tc.tile_pool(name="tpool", bufs=8) as tpool,
            tc.tile_pool(name="opool", bufs=2) as opool,
            tc.tile_pool(name="psum", bufs=1, space="PSUM") as ppool,
        ):
            xa_t = xpool.tile([PART, NBLK, XR, XC], bf, tag="xa")
            xb_t = xpool.tile([PART, NBLK, XR, XC], bf, tag="xb")
            id_t = cpool.tile([PART, PART], bf, tag="id")
            nc.sync.dma_start(xa_t[:], xa_d[:])
            nc.sync.dma_start(xb_t[:], xb_d[:])
            nc.sync.dma_start(id_t[:], id_d[:])

            acc = [
                [ppool.tile([PART, BANK], f32, name=f"ps{blk}_{j}", tag=f"ps{blk}_{j}") for j in range(NBANK)]
                for blk in range(NBLK)
            ]

            for kh in range(K):
                for kw in range(K):
                    k = kh * K + kw
                    # per-tap weight DMA (524KB) keeps startup latency low and
                    # interleaves smoothly with compute
                    w_t = wpool.tile([PART, 1, HHALF, W], bf, tag="w")
                    nc.sync.dma_start(
                        w_t[:], wr_d[:, k * HHALF * W : (k + 1) * HHALF * W]
                    )
                    # one DVE mult covers both channel blocks; the weight AP
                    # broadcasts (stride 0) over the block dim
                    tmp = tpool.tile([PART, NBLK, HHALF, W], bf, tag="tmp")
                    if kw % 2 == 0:
                        src = xa_t[:, :, kh : kh + HHALF, kw : kw + W]
                    else:
                        src = xb_t[:, :, kh : kh + HHALF, kw + 1 : kw + 1 + W]
                    wap = w_t[:, 0:1, :, :].broadcast_to((PART, NBLK, HHALF, W))
                    nc.vector.tensor_mul(tmp[:], src, wap)
                    for blk in range(NBLK):
                        for j in range(NBANK):
                            nc.tensor.matmul(
                                acc[blk][j][:],
                                id_t[:],
                                tmp[:, blk, j * YB : (j + 1) * YB, :],
                                start=(k == 0),
                                stop=(k == KK - 1),
                            )

            for blk in range(NBLK):
                for j in range(NBANK):
                    ost = opool.tile([PART, BANK], f32, tag="ost")
                    nc.scalar.copy(ost[:], acc[blk][j][:])
                    nc.sync.dma_start(
                        out_d[:, blk * HHALF * W + j * BANK : blk * HHALF * W + (j + 1) * BANK],
                        ost[:],
                    )

    _dedupe_ldweights(nc)
    nc.compile()
    return nc


def _dedupe_ldweights(nc):
    """All PE matmuls share one identity stationary; drop every InstLdweights
    after the first so the PE array keeps the loaded weights. Only removes
    LdWeights that carry no semaphore activity and whose AP matches the
    first one exactly."""
    first_repr = None
    removed = 0
    for blk in nc.main_func.blocks:
        keep = []
        for inst in blk.instructions:
            if type(inst).__name__ == "InstLdweights":
                si = inst.sync_info
                clean = si is None or (not si.on_wait and not si.on_update)
                r = repr(inst.ins[0])
                if first_repr is None:
                    first_repr = r
                elif clean and r == first_repr:
                    removed += 1
                    continue
            keep.append(inst)
        blk.instructions[:] = keep
    return removed


def _prep_core(x, w, b, h):
    """Host-side shard prep for one core: pad/cast x, slice/replicate w."""
    y0 = h * HHALF
    xa = np.zeros((C, XR, XC), dtype=np.float32)
    ylo, yhi = y0 - PAD, y0 + HHALF + PAD
    slo, shi = max(ylo, 0), min(yhi, H)
    xa[:, slo - ylo : shi - ylo, PAD : PAD + W] = x[b, :, slo:shi, :]
    xb = np.zeros((C, XR, XC), dtype=np.float32)
    xb[:, :, 1:] = xa[:, :, :-1]
    # partition-major: [128, NBLK, XR, XC], channel = blk*128 + p
    xa = xa.reshape(NBLK, PART, XR, XC).transpose(1, 0, 2, 3)
    xb = xb.reshape(NBLK, PART, XR, XC).transpose(1, 0, 2, 3)
    # weights: [128, 49, 32, 64], partition p uses weight channel p % 16
    wsh = w[b, 0, :, :, y0 : y0 + HHALF, :]  # [16, 49, 32, 64]
    wr = np.tile(wsh, (PART // WC, 1, 1, 1))  # [128, 49, 32, 64]
    return (
        xa.reshape(PART, -1).astype(_BF16),
        xb.reshape(PART, -1).astype(_BF16),
        wr.reshape(PART, -1).astype(_BF16),
    )


def kernel(x: np.ndarray, weight: np.ndarray) -> np.ndarray:
    from concourse.bass_utils import run_bass_kernel_spmd

    if "nc" not in _cache:
        _cache["nc"] = _build()
    nc = _cache["nc"]

    ident = np.eye(PART, dtype=_BF16)
    in_maps = []
    for core in range(NCORES):
        b, h = core // 2, core % 2
        xa, xb, wr = _prep_core(x, weight, b, h)
        in_maps.append({"xa": xa, "xb": xb, "wr": wr, "ident": ident})

    res = run_bass_kernel_spmd(nc, in_maps, list(range(NCORES)))

    out = np.empty((B, C, H, W), dtype=np.float32)
    for core in range(NCORES):
        b, h = core // 2, core % 2
        o = res.results[core]["out"].reshape(PART, NBLK, HHALF, W)
        out[b, :, h * HHALF : (h + 1) * HHALF, :] = o.transpose(1, 0, 2, 3).reshape(
            C, HHALF, W
        )
    return out



# revision 2
# speedup vs baseline: 1.0159x; 1.0159x over previous
import sys

for _p in ("/root/.axon_site/_ro/trn_rl_repo", "/opt/trn_rl_repo"):
    if _p not in sys.path:
        sys.path.insert(0, _p)

import numpy as np
import ml_dtypes

# Problem constants (nn_LocalConvolution): x [4,256,64,64] f32,
# weight [4,1,16,49,64,64] f32, K=7, pad=3, stride=1, dil=1.
# out[b, g*16+cc, y, x] = sum_k x_pad[b, g*16+cc, y+kh-3, x+kw-3] * w[b,0,cc,k,y,x]
B, C, H, W = 4, 256, 64, 64
WC, KK, K, PAD = 16, 49, 7, 3
G = C // WC  # 16 channel groups sharing each weight channel
NCORES = 8
HHALF = H // 2  # 32 output rows per core (B=4 x 2 H-halves = 8 shards)
PART = 128
# Partition p = cc*8 + yb: weight channel cc in [0,16), y-block yb in [0,8).
# Each partition computes 4 output rows (yq) x 16 groups (g) x 64 cols.
NYB = 8  # y-blocks per core
YQ = 4  # rows per y-block
ROWS = YQ + K - 1  # 10 halo rows of padded input per partition
XC = W + 2 * PAD  # 70 padded cols
FREE = YQ * G * W  # 4096 product elems per partition per tap
BANK = 512  # fp32 elems per PSUM bank
NBANK = FREE // BANK  # 8 banks

# Taps executed on GPSIMD instead of DVE (tune for balance).
GP_TAPS = frozenset()

_BF16 = ml_dtypes.bfloat16
_cache = {}


def _build():
    import concourse.bacc as bacc
    import concourse.mybir as mybir
    import concourse.tile as tile

    nc = bacc.Bacc(None, target_bir_lowering=False)
    bf = mybir.dt.bfloat16
    f32 = mybir.dt.float32

    xa_d = nc.dram_tensor("xa", (PART, ROWS * G * XC), bf, kind="ExternalInput")
    xb_d = nc.dram_tensor("xb", (PART, ROWS * G * XC), bf, kind="ExternalInput")
    wr_d = nc.dram_tensor("wr", (PART, KK * YQ * W), bf, kind="ExternalInput")
    id_d = nc.dram_tensor("ident", (PART, PART), bf, kind="ExternalInput")
    out_d = nc.dram_tensor("out", (PART, FREE), f32, kind="ExternalOutput")

    with tile.TileContext(nc) as tc:
        with (
            tc.tile_pool(name="xpool", bufs=1) as xpool,
            tc.tile_pool(name="cpool", bufs=1) as cpool,
            tc.tile_pool(name="wpool", bufs=4) as wpool,
            tc.tile_pool(name="tpool", bufs=6) as tpool,
            tc.tile_pool(name="gpool", bufs=2) as gpool,
            tc.tile_pool(name="opool", bufs=4) as opool,
            tc.tile_pool(name="psum", bufs=1, space="PSUM") as ppool,
        ):
            xa_t = xpool.tile([PART, ROWS, G, XC], bf, tag="xa")
            xb_t = xpool.tile([PART, ROWS, G, XC], bf, tag="xb")
            id_t = cpool.tile([PART, PART], bf, tag="id")
            nc.sync.dma_start(xa_t[:], xa_d[:])
            nc.sync.dma_start(xb_t[:], xb_d[:])
            nc.sync.dma_start(id_t[:], id_d[:])

            # weight chunks, one per kh row (7 taps each) to overlap DMA
            w_rows = []
            for kh in range(K):
                w_t = wpool.tile([PART, K, YQ, 1, W], bf, tag=f"w{kh}")
                nc.sync.dma_start(
                    w_t[:], wr_d[:, kh * K * YQ * W : (kh + 1) * K * YQ * W]
                )
                w_rows.append(w_t)

            acc = [
                ppool.tile([PART, BANK], f32, name=f"ps{j}", tag=f"ps{j}")
                for j in range(NBANK)
            ]

            for kh in range(K):
                for kw in range(K):
                    k = kh * K + kw
                    if kw % 2 == 0:
                        src = xa_t[:, kh : kh + YQ, :, kw : kw + W]
                    else:
                        src = xb_t[:, kh : kh + YQ, :, kw + 1 : kw + 1 + W]
                    wap = w_rows[kh][:, kw, :, :, :].broadcast_to((PART, YQ, G, W))
                    pool = gpool if k in GP_TAPS else tpool
                    tmp = pool.tile([PART, YQ, G, W], bf, tag="gtmp" if k in GP_TAPS else "tmp")
                    eng = nc.gpsimd if k in GP_TAPS else nc.vector
                    eng.tensor_mul(tmp[:], src, wap)
                    for j in range(NBANK):
                        yq, gh = j // 2, j % 2
                        nc.tensor.matmul(
                            acc[j][:],
                            id_t[:],
                            tmp[:, yq, gh * 8 : (gh + 1) * 8, :],
                            start=(k == 0),
                            stop=(k == KK - 1),
                        )

            for j in range(NBANK):
                ost = opool.tile([PART, BANK], f32, tag="ost")
                nc.scalar.copy(ost[:], acc[j][:])
                nc.sync.dma_start(out_d[:, j * BANK : (j + 1) * BANK], ost[:])

    _dedupe_ldweights(nc)
    nc.compile()
    return nc


def _dedupe_ldweights(nc):
    """All PE matmuls share one identity stationary; drop every InstLdweights
    after the first so the PE array keeps the loaded weights. Only removes
    LdWeights that carry no semaphore activity and whose AP matches the
    first one exactly."""
    first_repr = None
    removed = 0
    for blk in nc.main_func.blocks:
        keep = []
        for inst in blk.instructions:
            if type(inst).__name__ == "InstLdweights":
                si = inst.sync_info
                clean = si is None or (not si.on_wait and not si.on_update)
                r = repr(inst.ins[0])
                if first_repr is None:
                    first_repr = r
                elif clean and r == first_repr:
                    removed += 1
                    continue
            keep.append(inst)
        blk.instructions[:] = keep
    return removed


def _prep_core(x, w, b, h):
    """Host-side shard prep for one core: pad/slab x, reshape w.

    Layout: partition p = cc*8 + yb. xa[p] = padded rows
    [h*32+yb*4, +10) of channels {g*16+cc}, shape [ROWS, G, XC].
    xb = xa shifted right by one column (for odd-kw 4B alignment).
    wr[p] = weight[b,0,cc,:,h*32+yb*4:+4,:], shape [KK, YQ, W].
    """
    xpad = np.zeros((C, H + 2 * PAD, XC), dtype=np.float32)
    xpad[:, PAD : PAD + H, PAD : PAD + W] = x[b]
    v = xpad[:, h * HHALF : h * HHALF + HHALF + 2 * PAD, :]  # [C, 38, 70]
    vr = v.reshape(G, WC, HHALF + 2 * PAD, XC)  # [g, cc, 38, 70]
    xa = np.empty((WC, NYB, ROWS, G, XC), dtype=np.float32)
    for yb in range(NYB):
        # [g, cc, 10, 70] -> [cc, 10, g, 70]
        xa[:, yb] = vr[:, :, yb * YQ : yb * YQ + ROWS, :].transpose(1, 2, 0, 3)
    xa = xa.reshape(PART, -1)
    xb = np.zeros_like(xa)
    xb3 = xb.reshape(PART, ROWS * G, XC)
    xb3[:, :, 1:] = xa.reshape(PART, ROWS * G, XC)[:, :, :-1]
    # weights: [16, 49, 32, 64] -> [cc, yb, k, yq, x] -> [128, 49*4*64]
    wsh = w[b, 0, :, :, h * HHALF : (h + 1) * HHALF, :]
    wr = wsh.reshape(WC, KK, NYB, YQ, W).transpose(0, 2, 1, 3, 4)
    return (
        xa.astype(_BF16),
        xb.astype(_BF16),
        np.ascontiguousarray(wr).reshape(PART, -1).astype(_BF16),
    )


def _prep_inputs(x, weight):
    ident = np.eye(PART, dtype=_BF16)
    in_maps = []
    for core in range(NCORES):
        b, h = core // 2, core % 2
        xa, xb, wr = _prep_core(x, weight, b, h)
        in_maps.append({"xa": xa, "xb": xb, "wr": wr, "ident": ident})
    return in_maps


def _unpack_out(results):
    out = np.empty((B, C, H, W), dtype=np.float32)
    for core in range(NCORES):
        b, h = core // 2, core % 2
        # [cc, yb, yq, gh, g8, x] -> c = gh*128 + g8*16 + cc, y = yb*4+yq
        o = results[core]["out"].reshape(WC, NYB, YQ, 2, 8, W)
        o = o.transpose(3, 4, 0, 1, 2, 5).reshape(C, HHALF, W)
        out[b, :, h * HHALF : (h + 1) * HHALF, :] = o
    return out


def kernel(x: np.ndarray, weight: np.ndarray) -> np.ndarray:
    from concourse.bass_utils import run_bass_kernel_spmd

    if "nc" not in _cache:
        _cache["nc"] = _build()
    nc = _cache["nc"]

    in_maps = _prep_inputs(x, weight)
    res = run_bass_kernel_spmd(nc, in_maps, list(range(NCORES)))
    return _unpack_out(res.results)


# revision 5
# speedup vs baseline: 1.0662x; 1.0495x over previous
import sys

for _p in ("/root/.axon_site/_ro/trn_rl_repo", "/opt/trn_rl_repo"):
    if _p not in sys.path:
        sys.path.insert(0, _p)

import numpy as np
import ml_dtypes

# Problem constants (nn_LocalConvolution): x [4,256,64,64] f32,
# weight [4,1,16,49,64,64] f32, K=7, pad=3, stride=1, dil=1.
# out[b, g*16+cc, y, x] = sum_k x_pad[b, g*16+cc, y+kh-3, x+kw-3] * w[b,0,cc,k,y,x]
B, C, H, W = 4, 256, 64, 64
WC, KK, K, PAD = 16, 49, 7, 3
G = C // WC  # 16 channel groups sharing each weight channel
NCORES = 8
HHALF = H // 2  # 32 output rows per core (B=4 x 2 H-halves = 8 shards)
PART = 128
# Partition p = cc*8 + yb: weight channel cc in [0,16), y-block yb in [0,8).
# Each partition computes 4 output rows (yq) x 16 groups (g) x 64 cols.
NYB = 8  # y-blocks per core
YQ = 4  # rows per y-block
ROWS = YQ + K - 1  # 10 halo rows of padded input per partition
XC = W + 2 * PAD  # 70 padded cols
FREE = YQ * G * W  # 4096 product elems per partition per tap
BANK = 512  # fp32 elems per PSUM bank
NBANK = FREE // BANK  # 8 banks

# Tap processing order: even kw first (xb only needed for odd kw, so its
# DMA is off the critical path), kh-major so weight chunk kh=0 unblocks
# the first taps.
TAP_SEQ = [(kh, kw) for kh in range(K) for kw in (0, 2, 4, 6)] + [
    (kh, kw) for kh in range(K) for kw in (1, 3, 5)
]
# Positions in TAP_SEQ executed on GPSIMD instead of DVE (tune for balance).
GP_POS = frozenset()

_BF16 = ml_dtypes.bfloat16
_cache = {}


def _build():
    import concourse.bacc as bacc
    import concourse.mybir as mybir
    import concourse.tile as tile

    nc = bacc.Bacc(None, target_bir_lowering=False)
    bf = mybir.dt.bfloat16
    f32 = mybir.dt.float32

    xa_d = nc.dram_tensor("xa", (PART, ROWS * G * XC), bf, kind="ExternalInput")
    xb_d = nc.dram_tensor("xb", (PART, ROWS * G * XC), bf, kind="ExternalInput")
    wr_d = nc.dram_tensor("wr", (PART, KK * YQ * W), bf, kind="ExternalInput")
    id_d = nc.dram_tensor("ident", (PART, PART), bf, kind="ExternalInput")
    out_d = nc.dram_tensor("out", (PART, FREE), f32, kind="ExternalOutput")

    with tile.TileContext(nc) as tc:
        with (
            tc.tile_pool(name="xpool", bufs=1) as xpool,
            tc.tile_pool(name="cpool", bufs=1) as cpool,
            tc.tile_pool(name="wpool", bufs=4) as wpool,
            tc.tile_pool(name="tpool", bufs=6) as tpool,
            tc.tile_pool(name="gpool", bufs=2) as gpool,
            tc.tile_pool(name="opool", bufs=8) as opool,
            tc.tile_pool(name="psum", bufs=1, space="PSUM") as ppool,
        ):
            xa_t = xpool.tile([PART, ROWS, G, XC], bf, tag="xa")
            xb_t = xpool.tile([PART, ROWS, G, XC], bf, tag="xb")
            id_t = cpool.tile([PART, PART], bf, tag="id")
            warm = cpool.tile([PART, 1], f32, tag="warm")
            # critical path: ident then xa on the sync queue; weights on the
            # scalar queue in parallel; xb (odd-kw phase) last.
            nc.sync.dma_start(id_t[:], id_d[:])
            nc.sync.dma_start(xa_t[:], xa_d[:])
            nc.sync.dma_start(xb_t[:], xb_d[:])

            # weight chunks, one per kh row (7 taps each) to overlap DMA
            w_rows = []
            for kh in range(K):
                w_t = wpool.tile([PART, K, YQ, 1, W], bf, tag=f"w{kh}")
                nc.scalar.dma_start(
                    w_t[:], wr_d[:, kh * K * YQ * W : (kh + 1) * K * YQ * W]
                )
                w_rows.append(w_t)

            # preload the ACT copy table set during the head so the tail's
            # PSUM->SBUF copies don't pay ACT_TABLE_LOAD
            nc.scalar.copy(warm[:], id_t[:, 0:1])

            acc = [
                ppool.tile([PART, BANK], f32, name=f"ps{j}", tag=f"ps{j}")
                for j in range(NBANK)
            ]

            def tap_src(kh, kw):
                if kw % 2 == 0:
                    src = xa_t[:, kh : kh + YQ, :, kw : kw + W]
                else:
                    src = xb_t[:, kh : kh + YQ, :, kw + 1 : kw + 1 + W]
                wap = w_rows[kh][:, kw, :, :, :].broadcast_to((PART, YQ, G, W))
                return src, wap

            gp_pos = sorted(GP_POS)
            gp_tmp = {}

            def gp_issue(pos):
                kh, kw = TAP_SEQ[pos]
                src, wap = tap_src(kh, kw)
                tmp = gpool.tile([PART, YQ, G, W], bf, tag="gtmp")
                nc.gpsimd.tensor_mul(tmp[:], src, wap)
                gp_tmp[pos] = tmp

            if gp_pos:
                gp_issue(gp_pos[0])

            for pos, (kh, kw) in enumerate(TAP_SEQ):
                if pos in GP_POS:
                    # product was issued one gp-slot ago; prefetch the next
                    i = gp_pos.index(pos)
                    if i + 1 < len(gp_pos):
                        gp_issue(gp_pos[i + 1])
                    tmp = gp_tmp.pop(pos)
                else:
                    src, wap = tap_src(kh, kw)
                    tmp = tpool.tile([PART, YQ, G, W], bf, tag="tmp")
                    nc.vector.tensor_mul(tmp[:], src, wap)
                for j in range(NBANK):
                    yq, gh = j // 2, j % 2
                    nc.tensor.matmul(
                        acc[j][:],
                        id_t[:],
                        tmp[:, yq, gh * 8 : (gh + 1) * 8, :],
                        start=(pos == 0),
                        stop=(pos == KK - 1),
                    )

            for j in range(NBANK):
                ost = opool.tile([PART, BANK], f32, tag="ost")
                eng = nc.scalar if j % 2 == 0 else nc.vector
                if j % 2 == 0:
                    eng.copy(ost[:], acc[j][:])
                else:
                    eng.tensor_copy(ost[:], acc[j][:])
                nc.sync.dma_start(out_d[:, j * BANK : (j + 1) * BANK], ost[:])

    _dedupe_ldweights(nc)
    nc.compile()
    return nc


def _dedupe_ldweights(nc):
    """All PE matmuls share one identity stationary; drop every InstLdweights
    after the first so the PE array keeps the loaded weights. Only removes
    LdWeights that carry no semaphore activity and whose AP matches the
    first one exactly."""
    first_repr = None
    removed = 0
    for blk in nc.main_func.blocks:
        keep = []
        for inst in blk.instructions:
            if type(inst).__name__ == "InstLdweights":
                si = inst.sync_info
                clean = si is None or (not si.on_wait and not si.on_update)
                r = repr(inst.ins[0])
                if first_repr is None:
                    first_repr = r
                elif clean and r == first_repr:
                    removed += 1
                    continue
            keep.append(inst)
        blk.instructions[:] = keep
    return removed


def _prep_core(x, w, b, h):
    """Host-side shard prep for one core: pad/slab x, reshape w.

    Layout: partition p = cc*8 + yb. xa[p] = padded rows
    [h*32+yb*4, +10) of channels {g*16+cc}, shape [ROWS, G, XC].
    xb = xa shifted right by one column (for odd-kw 4B alignment).
    wr[p] = weight[b,0,cc,:,h*32+yb*4:+4,:], shape [KK, YQ, W].
    """
    xpad = np.zeros((C, H + 2 * PAD, XC), dtype=np.float32)
    xpad[:, PAD : PAD + H, PAD : PAD + W] = x[b]
    v = xpad[:, h * HHALF : h * HHALF + HHALF + 2 * PAD, :]  # [C, 38, 70]
    vr = v.reshape(G, WC, HHALF + 2 * PAD, XC)  # [g, cc, 38, 70]
    xa = np.empty((WC, NYB, ROWS, G, XC), dtype=np.float32)
    for yb in range(NYB):
        # [g, cc, 10, 70] -> [cc, 10, g, 70]
        xa[:, yb] = vr[:, :, yb * YQ : yb * YQ + ROWS, :].transpose(1, 2, 0, 3)
    xa = xa.reshape(PART, -1)
    xb = np.zeros_like(xa)
    xb3 = xb.reshape(PART, ROWS * G, XC)
    xb3[:, :, 1:] = xa.reshape(PART, ROWS * G, XC)[:, :, :-1]
    # weights: [16, 49, 32, 64] -> [cc, yb, k, yq, x] -> [128, 49*4*64]
    wsh = w[b, 0, :, :, h * HHALF : (h + 1) * HHALF, :]
    wr = wsh.reshape(WC, KK, NYB, YQ, W).transpose(0, 2, 1, 3, 4)
    return (
        xa.astype(_BF16),
        xb.astype(_BF16),
        np.ascontiguousarray(wr).reshape(PART, -1).astype(_BF16),
    )


def _prep_inputs(x, weight):
    ident = np.eye(PART, dtype=_BF16)
    in_maps = []
    for core in range(NCORES):
        b, h = core // 2, core % 2
        xa, xb, wr = _prep_core(x, weight, b, h)
        in_maps.append({"xa": xa, "xb": xb, "wr": wr, "ident": ident})
    return in_maps


def _unpack_out(results):
    out = np.empty((B, C, H, W), dtype=np.float32)
    for core in range(NCORES):
        b, h = core // 2, core % 2
        # [cc, yb, yq, gh, g8, x] -> c = gh*128 + g8*16 + cc, y = yb*4+yq
        o = results[core]["out"].reshape(WC, NYB, YQ, 2, 8, W)
        o = o.transpose(3, 4, 0, 1, 2, 5).reshape(C, HHALF, W)
        out[b, :, h * HHALF : (h + 1) * HHALF, :] = o
    return out


def kernel(x: np.ndarray, weight: np.ndarray) -> np.ndarray:
    from concourse.bass_utils import run_bass_kernel_spmd

    if "nc" not in _cache:
        _cache["nc"] = _build()
    nc = _cache["nc"]

    in_maps = _prep_inputs(x, weight)
    res = run_bass_kernel_spmd(nc, in_maps, list(range(NCORES)))
    return _unpack_out(res.results)
